# revision 1
# baseline (speedup 1.0000x reference)
"""Trainium2 Bass kernel for nn_MultiHeadAttention_47115791237226.

Computation (per token t):
  q,k,v = x @ {Wq,Wk,Wv}.T           (three 2048x2048 linears)
  reshape to (H=16, Dh=128) heads
  A[h,g] = q_h . k_g  (over Dh), causal tril mask on (h,g), softmax over g
  out[h] = sum_g A[h,g] v_g

Sharding: data-parallel over the 16384 tokens -> 2048 tokens per core, no
collectives. The linears run on the tensor engine at its fp16 roofline
(~41.4us per 128-token tile, 24 output chunks of 256 over concat(q,k,v));
the per-token head attention (135 score dots + 135 AV updates per tile)
is load-balanced across DVE, Act and Pool so it hides behind the matmuls.

Schedule: a software pipeline one W-group (4 tiles) deep.
 - Window g streams W once for 4 tiles; PSUM->SBUF copies go to per-tile
   qk[ P,4096 ] (oc<16) and v[ P,2048 ] (oc>=16) pools on Act.
 - Scores for group g are emitted inside window g as each k chunk's
   copies land (oc>=8): ~102 pairs as fused DVE STT+accum (194ns), ~33
   as one batched Pool broadcast-mult per (g, high-h run) whose per-pair
   Act Copy+accum_out reduces (479ns incl. accumulator read) drain on a
   deferred schedule through block 20.
 - exp (Act, no max-subtraction: |scores| <~60 so fp32 exp cannot
   overflow), 1/sum normalize (DVE), AV and the out-DMA of group g all
   run inside window g+1, interleaved per oc block. AV: heads 15,12,2
   accumulate as fused DVE STT chains; the rest split mul (Pool
   ts_mul / Act activation-scale, 6:4) + DVE 2x-mode TT add; all inits
   on Pool. The last group's attention runs inline in its own window's
   v-blocks (nothing left for PE to lose there).

Hard-won scheduling rules encoded here:
 - Act's in-order stream must never queue an op whose deps resolve late
   ahead of PSUM->SBUF copies: exp is emitted after block 0's copies,
   AV muls/reduces are sliced evenly across blocks, and v-chunk copies
   never sit behind softmax-dependent work.
 - Throwaway outputs (score STT `out`, reduce `out`) rotate through 8
   trash tiles each; a single trash tile serializes the whole stream on
   its write-ack WAW (~290ns/op instead of 194).
 - x tiles stream per group through the Act HWDGE queue (Pool's SWDGE
   backlog made prefetches late); out stores likewise.
 - W chunks stream through a bufs=2 pool on the SP/HWDGE engine as 4
   sub-DMAs per chunk; SP carries no other DMAs, so the round-robin
   HWDGE queue assignment advances exactly 8 between a slot's old and
   new writer -> same queue -> the WAW edge is program order and only
   the single PE-consumer wait remains.

kernel() accepts the FULL unsharded inputs and returns the FULL output.
"""

import os
import sys

import numpy as np

sys.path.insert(0, "/opt/trn_rl_repo")

import concourse.bass as bass  # noqa: E402
import concourse.mybir as mybir  # noqa: E402
import concourse.tile as tile  # noqa: E402

# Problem constants (hardcoded per contest rules)
DIMS = 2048
HEADS = 16
HD = DIMS // HEADS  # 128
B, L = 4, 4096
TOK = B * L  # 16384
NCORES = 8
TPC = TOK // NCORES  # 2048 tokens per core
P = 128  # SBUF partitions
DC = DIMS // P  # 16 contraction chunks
OC = 256  # output-dim chunk per PSUM tile
NOC = 3 * DIMS // OC  # 24 chunks across concat(q,k,v); q:0-7 k:8-15 v:16-23
HPC = OC // HD  # 2 heads per oc chunk
TG = 4  # token tiles per group (W is re-streamed once per group)
WSUB = 4  # sub-DMAs per W chunk; WSUB * wp-bufs must be == 0 mod 8

F16 = mybir.dt.float16
F32 = mybir.dt.float32
ALU = mybir.AluOpType
ACTF = mybir.ActivationFunctionType

# ---- static attention plan -------------------------------------------------
# Cost constants (ns engine-busy per op) from the TRN2 cost model.
_SD = float(os.environ.get("K_SD", 194.0))  # DVE fused score pair
_STT = float(os.environ.get("K_STT", 194.0))  # DVE fused AV pair
_ADD_D = float(os.environ.get("K_ADD_D", 127.0))  # DVE TT add (2x)
_ADD_P = float(os.environ.get("K_ADD_P", 349.0))  # Pool TT add
_TTM_P = float(os.environ.get("K_TTM_P", 349.0))  # Pool TT mult
_MUL_A = float(os.environ.get("K_MUL_A", 292.0))  # Act mul (ptr scale)
_MUL_P = float(os.environ.get("K_MUL_P", 273.0))  # Pool tensor_scalar_mul
_D0 = float(os.environ.get("K_D0", 1712.0))  # DVE softmax fixed
_A0 = float(os.environ.get("K_A0", 13614.0))  # Act copies+exp+inits fixed
_P0 = float(os.environ.get("K_P0", 2100.0))  # Pool SWDGE fixed

# Heads whose AV accumulation chain lives on DVE; the rest chain on Pool.
_DVE_CHAIN = set(
    int(x) for x in os.environ.get("K_DVE_CHAIN", "1,2,13,14,15").split(",")
)


# Op menu (per attention pair, [P,128] fp16):
#  scores: 'sD' DVE fused STT+accum (194) | 'sP' Pool TT-mult (349) + Act
#          Copy+accum_out reduce (292), reduce deferred 2 oc blocks
#  AV:     'stt' DVE fused (194) | 'pa' Act mul (292) + Pool add (349)
#  init:   tensor_scalar_mul on Pool (273)
# Quotas from an LP over the cost model: per tile DVE 34.7us / Act 34.1 /
# Pool 34.3, i.e. ~139us per 166us matmul window per engine.
_N_SP = int(os.environ.get("K_N_SP", 33))  # of 135 score pairs on sP
_RUNCAP = int(os.environ.get("K_RUNCAP", 3))  # max heads per batched sP mult
_FLUSH_OC = int(os.environ.get("K_FLUSH_OC", 16))  # first flush emission block
# AV: heads whose chains run fused on DVE (sum of h = stt quota 85);
# the rest ('pa') run Act-mul + Pool-add. Chains stay whole per head so
# the two emission regions preserve per-chain g-order.
_STT_HEADS = set(
    int(x) for x in os.environ.get("K_STT_HEADS", "15,12,2").split(",")
)
_ADD_HEADS = set(
    int(x) for x in os.environ.get("K_ADD_HEADS", "14,13,11,10,9,8,7,6,5,4,3,1").split(",")
)
_DCLASS = _STT_HEADS | _ADD_HEADS  # chains whose adds run on DVE
# within ADD_HEADS, mul engine alternates Pool:Act at this ratio
_DP_RATIO = tuple(
    int(x) for x in os.environ.get("K_DP_RATIO", "6,4").split(",")
)


def _plan_scores():
    """Per g, cluster the sP quota into a contiguous high-h run (one batched
    Pool mult per (g, run)); everything else is sD."""
    plan = {}
    runs = {}
    total = 135
    acc = 0.0
    for g in range(HEADS):
        h0 = max(g, 1)
        npairs = HEADS - h0
        acc += _N_SP * npairs / total
        q = int(acc)
        acc -= q
        q = min(q, npairs, _RUNCAP)
        lo = HEADS - q
        runs[g] = (lo, HEADS - 1) if q > 0 else None
        for h in range(h0, HEADS):
            plan[(h, g)] = "sP" if h >= lo else "sD"
    return plan, runs


_SCORE_PLAN, _SP_RUNS = _plan_scores()


def _emit_scores(nc, prodp, deferred, qt, s, trashes, g_lo, g_hi,
                 only_sp=False, only_sd=False):
    """Score pairs for g in [g_lo, g_hi]: sD fused on DVE; the sP h-run as
    one batched Pool broadcast-mult, its per-pair Act reduces appended to
    `deferred` as (prod2d, scol). The throwaway STT outputs rotate through
    `trashes` -- a single trash tile would serialize the stream on its
    write-ack WAW (~290ns/op instead of 194)."""
    for g in range(g_lo, g_hi + 1):
        h0 = max(g, 1)
        if not only_sp:
            for h in range(h0, HEADS):
                if _SCORE_PLAN[(h, g)] != "sD":
                    continue
                nc.vector.scalar_tensor_tensor(
                    out=trashes[(h + g * HEADS) % len(trashes)],
                    in0=qt[:, h * HD : (h + 1) * HD],
                    scalar=1.0,
                    in1=qt[:, DIMS + g * HD : DIMS + (g + 1) * HD],
                    op0=ALU.bypass,
                    op1=ALU.mult,
                    accum_out=s[:, h * HEADS + g : h * HEADS + g + 1],
                )
        if only_sd:
            continue
        run = _SP_RUNS[g]
        if run is None:
            continue
        lo, hi = run
        n = hi - lo + 1
        prod = prodp.tile([P, n * HD], F16, tag="prod")
        kg = qt[:, DIMS + g * HD : DIMS + (g + 1) * HD]
        nc.gpsimd.tensor_tensor(
            out=prod.rearrange("p (n d) -> p n d", d=HD),
            in0=qt[:, lo * HD : (hi + 1) * HD].rearrange("p (n d) -> p n d", d=HD),
            in1=kg[:, None, :].to_broadcast((P, n, HD)),
            op=ALU.mult,
        )
        for i in range(n):
            h = lo + i
            deferred.append(
                (prod[:, i * HD : (i + 1) * HD],
                 s[:, h * HEADS + g : h * HEADS + g + 1])
            )


_red_i = [0]


def _emit_deferred_reduces(nc, deferred, trashes_a, n):
    """Emit up to n pending Act accum-reduces (rotating trash outs)."""
    for prod, scol in deferred[:n]:
        _red_i[0] += 1
        nc.scalar.activation(
            trashes_a[_red_i[0] % len(trashes_a)], prod, ACTF.Copy,
            accum_out=scol,
        )
    del deferred[:n]


# Flush (last group) head classes: the tail's engine release order is
# Act (~120us into the last window) < Pool (~130) < DVE (~143, after the
# score spill), so the flush leans on 'pa' pairs (Act mul + Pool add --
# fine here, no future copies to block) and fused stt on DVE.
_F_STT = set(int(x) for x in os.environ.get("K_F_STT", "15,12,2").split(","))
_F_PA = set(int(x) for x in os.environ.get("K_F_PA", "99").split(","))
_F_DPRATIO = tuple(
    int(x) for x in os.environ.get("K_F_DPRATIO", "6,4").split(",")
)


def _av_region_ops(dclass):
    """Two op lists (D-region, P-region), each g-major round-robin across
    tiles, chains whole per head. Entries: (t, kind, h, g)."""
    dops, pops = [], []
    for g in range(HEADS):
        for t in range(TG):
            if g == 0:
                pops.append((t, "head0", 0, 0))
                for h in range(1, HEADS):
                    (dops if h in dclass else pops).append((t, "init", h, 0))
            else:
                for h in range(HEADS - 1, g - 1, -1):
                    (dops if h in dclass else pops).append((t, "pair", h, g))
    return dops, pops


_D_OPS, _P_OPS = _av_region_ops(_DCLASS)


def _chop(n_ops, nslices):
    return [n_ops * i // nslices for i in range(nslices + 1)]


# D-region over oc blocks 0..7 (fills DVE before its scores can start),
# P-region over blocks 0..23.
_D_BOUNDS = _chop(len(_D_OPS), 8)
_P_BOUNDS = _chop(len(_P_OPS), NOC)


def _emit_av_ops(nc, avp, prev, ops, flush=False):
    """Emit AV ops. Steady plan: stt heads fused on DVE, the rest split
    mul (Pool/Act) + DVE add, inits on Pool. Flush plan (last group):
    its own head classes incl. 'pa' (Act mul + Pool add), inits split."""
    vts, ss, accs = prev["vts"], prev["ss"], prev["accs"]
    stt_heads = _F_STT if flush else _STT_HEADS
    pa_heads = _F_PA if flush else frozenset()
    dpr = _F_DPRATIO if flush else _DP_RATIO
    for t, kind, h, g in ops:
        vt = vts[t]
        s = ss[t]
        acc = accs[t]
        if kind == "head0":
            nc.scalar.copy(acc[:, 0:HD], vt[:, 0:HD])
            continue
        ah = acc[:, h * HD : (h + 1) * HD]
        if kind == "init":
            if flush and h % 2 == 0:
                nc.scalar.activation(
                    ah, vt[:, 0:HD], ACTF.Copy,
                    scale=s[:, h * HEADS : h * HEADS + 1],
                )
            else:
                nc.gpsimd.tensor_scalar_mul(
                    ah, vt[:, 0:HD], s[:, h * HEADS : h * HEADS + 1]
                )
            continue
        pcol = s[:, h * HEADS + g : h * HEADS + g + 1]
        vs = vt[:, g * HD : (g + 1) * HD]
        if h in stt_heads:
            nc.vector.scalar_tensor_tensor(
                out=ah, in0=vs, scalar=pcol, in1=ah, op0=ALU.mult, op1=ALU.add
            )
        elif h in pa_heads:
            tmp = avp.tile([P, HD], F16, tag="avtmp")
            nc.scalar.activation(tmp, vs, ACTF.Copy, scale=pcol)
            nc.gpsimd.tensor_tensor(out=ah, in0=ah, in1=tmp, op=ALU.add)
        else:
            tmp = avp.tile([P, HD], F16, tag="avtmp")
            ratio = dpr[0] + dpr[1]
            if (h * HEADS + g + 1) % ratio < dpr[0]:
                nc.gpsimd.tensor_scalar_mul(tmp, vs, pcol)
            else:
                nc.scalar.activation(tmp, vs, ACTF.Copy, scale=pcol)
            nc.vector.tensor_tensor(out=ah, in0=ah, in1=tmp, op=ALU.add)


def _emit_exp(nc, prev, tiles=None):
    for t in tiles if tiles is not None else range(TG):
        s = prev["ss"][t]
        nc.scalar.activation(s, s, ACTF.Exp)


def _emit_softmax_head(nc, accp, prev, tiles=None):
    """Sum/recip/normalize (DVE) for the previous group after its exp.
    Also allocates the group's acc tiles (first written by the inits in
    this window)."""
    for t in tiles if tiles is not None else range(TG):
        s = prev["ss"][t]
        s3 = s.rearrange("p (h g) -> p h g", g=HEADS)
        sumE = prev["sm"][t][:, 0:HEADS]
        recip = prev["sm"][t][:, HEADS : 2 * HEADS]
        nc.vector.tensor_reduce(sumE, s3, axis=mybir.AxisListType.X, op=ALU.add)
        nc.vector.reciprocal(recip, sumE)
        nc.vector.tensor_tensor(
            out=s3,
            in0=s3,
            in1=recip[:, :, None].to_broadcast((P, HEADS, HEADS)),
            op=ALU.mult,
        )
    if "accs" not in prev:
        prev["accs"] = [
            accp.tile([P, DIMS], F16, name="acc", tag="acc") for _ in range(TG)
        ]


def _emit_out(nc, out, prev):
    for t in range(TG):
        ti = prev["gi"] * TG + t
        nc.scalar.dma_start(out[ti * P : (ti + 1) * P, :], prev["accs"][t])


def _body(tc, xt, wt, out, nt, reps=1):
    nc = tc.nc
    ngroups_real = nt // TG
    ngroups = ngroups_real * reps
    dsub = DC // WSUB
    nD, nP = len(_D_OPS), len(_P_OPS)
    # last-group inline flush: per v-block ops, both regions merged
    flush_by_g = {}
    for op in _D_OPS + _P_OPS:
        flush_by_g.setdefault(op[3], []).append(op)
    with (
        tc.tile_pool(name="xp", bufs=2 * TG - 2) as xp,
        tc.tile_pool(name="wp", bufs=2) as wp,
        tc.tile_pool(name="qkp", bufs=2 * TG) as qkp,
        tc.tile_pool(name="vp", bufs=2 * TG) as vp,
        tc.tile_pool(name="psum", bufs=8, space="PSUM") as pp,
        tc.tile_pool(name="sp", bufs=2 * TG) as sp,
        tc.tile_pool(name="smallp", bufs=2 * TG) as smallp,
        tc.tile_pool(name="accp", bufs=2 * TG) as accp,
        tc.tile_pool(name="avp", bufs=48) as avp,
        tc.tile_pool(name="prodp", bufs=12) as prodp,
        tc.tile_pool(name="trashp", bufs=1) as trashp,
    ):
        trashes = [
            trashp.tile([P, HD], F16, name="trash_d", tag=f"trash_d{i}")
            for i in range(8)
        ]
        trashes_a = [
            trashp.tile([P, HD], F16, name="trash_a", tag=f"trash_a{i}")
            for i in range(8)
        ]
        xg_next = [xp.tile([P, DC * P], F16, name="xg", tag="xg") for _ in range(TG)]
        for t in range(TG):
            nc.scalar.dma_start(xg_next[t], xt[t])
        prev = None
        for gi_r in range(ngroups):
            gi = gi_r % ngroups_real
            is_last = gi_r == ngroups - 1
            xg = xg_next
            if not is_last:
                gn = (gi_r + 1) % ngroups_real
                xg_next = [
                    xp.tile([P, DC * P], F16, name="xg", tag="xg")
                    for _ in range(TG)
                ]
                for t in range(TG):
                    nc.scalar.dma_start(xg_next[t], xt[gn * TG + t])
            qks = [
                qkp.tile([P, 2 * DIMS], F16, name="qk", tag="qk")
                for _ in range(TG)
            ]
            vts = [
                vp.tile([P, DIMS], F16, name="v", tag="v") for _ in range(TG)
            ]
            ss = [
                sp.tile([P, HEADS * HEADS], F32, name="s", tag="s")
                for _ in range(TG)
            ]
            sm = [
                smallp.tile([P, 2 * HEADS], F32, name="sm", tag="sm")
                for _ in range(TG)
            ]
            cur = {"gi": gi, "qks": qks, "vts": vts, "ss": ss, "sm": sm}
            for t in range(TG):
                nc.vector.memset(ss[t], -30000.0)
            deferred = []
            for oc in range(NOC):
                w = wp.tile([P, DC, OC], F16, tag="w")
                for sub in range(WSUB):
                    nc.sync.dma_start(
                        w[:, sub * dsub : (sub + 1) * dsub, :],
                        wt[:, sub * dsub : (sub + 1) * dsub, oc * OC : (oc + 1) * OC],
                    )
                for t in range(TG):
                    ps = pp.tile([P, OC], F32, tag="ps")
                    for d in range(DC):
                        nc.tensor.matmul(
                            ps,
                            lhsT=xg[t][:, d * P : (d + 1) * P],
                            rhs=w[:, d, :],
                            start=(d == 0),
                            stop=(d == DC - 1),
                        )
                    if oc < 16:
                        nc.scalar.copy(qks[t][:, oc * OC : (oc + 1) * OC], ps)
                    else:
                        nc.scalar.copy(
                            vts[t][:, (oc - 16) * OC : (oc - 15) * OC], ps
                        )
                # the previous group's softmax tail (exp, then 1/sum
                # normalize) goes after block 0's copies so a late score
                # tail can never head-of-line-block them; exp MUST be
                # emitted before the DVE sum/normalize (emission order is
                # program order for the in-place s update)
                if prev is not None and oc == 0:
                    _emit_exp(nc, prev)
                    _emit_softmax_head(nc, accp, prev)
                # last group's flush: exp's scores dependency resolves at
                # ~the same time block 21's copies become ready, so
                # emitting the flush from block 21 starts it just as early
                # as block 16 would, WITHOUT sitting ahead of copies
                # 17..21 that free PSUM for PE's final chunks.
                if is_last and oc >= _FLUSH_OC:
                    if oc == _FLUSH_OC:
                        _emit_exp(nc, cur)
                        _emit_softmax_head(nc, accp, cur)
                        g0, g1 = 0, (oc - 15) * HPC - 1
                    else:
                        g0, g1 = (oc - 16) * HPC, (oc - 15) * HPC - 1
                    for gg in range(g0, min(g1 + 1, 14)):
                        _emit_av_ops(
                            nc, avp, cur, flush_by_g.get(gg, []), flush=True
                        )
                # previous group's AV: DVE region in blocks 0..7, the
                # Act-mul/Pool-add region across all 24
                if prev is not None:
                    if oc < 8:
                        _emit_av_ops(
                            nc, avp, prev,
                            _D_OPS[_D_BOUNDS[oc] : _D_BOUNDS[oc + 1]],
                        )
                    _emit_av_ops(
                        nc, avp, prev, _P_OPS[_P_BOUNDS[oc] : _P_BOUNDS[oc + 1]]
                    )
                # this group's scores as the k chunks land
                if 8 <= oc <= 15:
                    g_lo, g_hi = (oc - 8) * HPC, (oc - 8) * HPC + HPC - 1
                    for t in range(TG):
                        _emit_scores(
                            nc, prodp, deferred, qks[t], ss[t], trashes,
                            g_lo, g_hi,
                        )
                    if is_last:
                        _emit_deferred_reduces(nc, deferred, trashes_a, len(deferred))
                    elif oc >= 9 and deferred:
                        nleft = 19 - oc
                        _emit_deferred_reduces(
                            nc, deferred, trashes_a, -(-len(deferred) // nleft)
                        )
                elif oc > 15 and deferred:
                    # drain pending sP reduces by block 18 so the last
                    # copies (PSUM wrap for the next group) are never
                    # stuck behind them on Act
                    nleft = max(19 - oc, 1)
                    _emit_deferred_reduces(
                        nc, deferred, trashes_a, -(-len(deferred) // nleft)
                    )
            if prev is not None:
                _emit_out(nc, out, prev)
            prev = cur
        for gg in range(14, HEADS):
            _emit_av_ops(nc, avp, prev, flush_by_g.get(gg, []), flush=True)
        _emit_out(nc, out, prev)


def build(tpc=TPC, reps=1):
    import concourse.bacc as bacc

    # Bacc (not raw Bass): its compile() pass splits multi-semaphore DMA
    # waits that the one-wait-slot DMA ISA encoding cannot carry.
    nc = bacc.Bacc(
        None,
        target_bir_lowering=False,
        debug=False,
        enable_asserts=True,
        num_devices=NCORES,
    )
    nt = tpc // P
    xt = nc.dram_tensor("xt", [nt, P, DC * P], F16, kind="ExternalInput").ap()
    wt = nc.dram_tensor("wt", [P, DC, 3 * DIMS], F16, kind="ExternalInput").ap()
    out = nc.dram_tensor("out", [tpc, DIMS], F16, kind="ExternalOutput").ap()
    with tile.TileContext(nc) as tc:
        _body(tc, xt, wt, out, nt, reps=reps)
    nc.compile()
    return nc


def prep_inputs(input_seq_embs, Wq, Wk, Wv, tpc=TPC, ncores=NCORES):
    """Host-side sharding + layout."""
    x = np.asarray(input_seq_embs, dtype=np.float32).reshape(TOK, DIMS)
    wall = np.concatenate(
        [np.asarray(Wq), np.asarray(Wk), np.asarray(Wv)], axis=0
    ).astype(np.float32)  # [3*DIMS, DIMS], row o, col d
    # wt[p, d, o] = wall[o, d*P+p]
    wt = np.ascontiguousarray(
        wall.T.reshape(DC, P, 3 * DIMS).transpose(1, 0, 2)
    ).astype(np.float16)
    in_maps = []
    for c in range(ncores):
        xs = x[c * tpc : (c + 1) * tpc]
        nt = tpc // P
        # xtile[t, p, d*P+q] = xs[t*P+q, d*P+p]
        xtile = (
            xs.reshape(nt, P, DC, P).transpose(0, 3, 2, 1).astype(np.float16)
        ).reshape(nt, P, DC * P)
        in_maps.append({"xt": np.ascontiguousarray(xtile), "wt": wt})
    return in_maps


_cached = {}


def _get_nc():
    if "nc" not in _cached:
        _cached["nc"] = build()
    return _cached["nc"]


def kernel_with_results(**inputs):
    from concourse import bass_utils

    nc = _get_nc()
    in_maps = prep_inputs(**inputs)
    trace = bool(int(os.environ.get("KERNEL_TRACE", "0")))
    if trace:
        try:  # NTFF profiling hook is absent in some containers
            from antenv.axon_hooks import get_axon_ntff_profile_hook  # noqa: F401
        except ImportError:
            trace = False
    res = bass_utils.run_bass_kernel_spmd(
        nc,
        in_maps,
        core_ids=list(range(NCORES)),
        trace=trace,
        trace_cores=[0] if trace else None,
    )
    outs = [r["out"] for r in res.results]
    full = (
        np.concatenate(outs, axis=0)
        .astype(np.float32)
        .reshape(B, L, DIMS)
    )
    return full, res


def kernel(**inputs):
    return kernel_with_results(**inputs)[0]



# revision 8
# speedup vs baseline: 3068.3681x; 3068.3681x over previous
"""Trainium2 Bass kernel v2 for nn_MultiHeadAttention_47115791237226.

Differences vs v1 (kernel_baseline.py):
 - q head 0 is never computed (row h=0 of A is fully masked except A[0,0],
   whose softmax is 1 regardless -> out head 0 = v head 0). W' has 6016
   output columns; the q1 chunk is 128 wide -> 2% less PE work.
 - qk chunk order interleaves Q (descending head pairs) with K (ascending):
   [Q7 K0 Q6 K1 ... Q1 K6 Q0' K7] then V0..V7. Score pair (h,g) becomes
   ready at max(qblk(h), kblk(g)) -> score work spreads over blocks 1..15
   instead of 8..15, and heavy rows (high h) are ready earliest.
 - a global static scheduler places every attention op (score units, exp,
   softmax, AV units, copies, out-DMA) into (window, block) slots against
   per-engine capacity ledgers. AV for group g starts inside window g's
   v-region ("early AV"); what doesn't fit spills into the next window's
   early blocks; only the last window's small spill runs post-matmul.
 - AV adds are batched: n pair-muls (Act/Pool) write one tmp [P,n*128],
   then a single DVE 2x TT add updates acc[:, h0*128:(h0+n)*128].
 - score units: 2-row cohorts -> fused DVE STT+accum (sD); >=3-row runs ->
   one broadcast mult (DVE or Pool) + one DVE segmented tensor_reduce.

s-tile rows are in PHYSICAL q order (phys2h = 14,15,12,13,...,2,3,1);
softmax is row-order independent; AV reads map h -> row.

kernel() accepts the FULL unsharded inputs and returns the FULL output.
"""

import os
import sys

import numpy as np

sys.path.insert(0, "/opt/trn_rl_repo")

import concourse.mybir as mybir  # noqa: E402
import concourse.tile as tile  # noqa: E402

DIMS = 2048
HEADS = 16
HD = DIMS // HEADS  # 128
B, L = 4, 4096
TOK = B * L
NCORES = 8
TPC = TOK // NCORES  # 2048
P = 128
DC = DIMS // P  # 16
TG = 4  # tiles per W window
WSUB = int(os.environ.get("K2_WSUB", "4"))  # sub-DMAs per W chunk
NROW = 15  # score rows (heads 1..15)
QW = NROW * HD  # 1920 q-region width
KW = DIMS  # 2048
WCOLS = QW + 2 * DIMS  # 6016

F16 = mybir.dt.float16
F32 = mybir.dt.float32
ALU = mybir.AluOpType
ACTF = mybir.ActivationFunctionType

# ---------------- chunk / block tables ----------------
# qk blocks 0..15: even b -> Q chunk (descending), odd b -> K chunk (asc).
# blocks 16..23: V chunks ascending.
# Each entry: (kind, heads, width, wofs, dofs) where wofs = column offset in
# the host-packed W' (block-major), dofs = dest offset in qk/v SBUF region.


def _build_blocks():
    qchunks = []  # descending: Q7=(14,15) ... Q1=(2,3), Q0'=(1,)
    for c in range(7, 0, -1):
        qchunks.append(((2 * c, 2 * c + 1), 256))
    qchunks.append(((1,), 128))
    blocks = []
    phys2h = []
    qofs = {}
    dq = 0
    for i in range(8):
        heads, wid = qchunks[i]
        blocks.append(["q", heads, wid, 0, dq])
        for h in heads:
            qofs[h] = dq + (len(qofs) - len(qofs)) * 0  # placeholder
        o = dq
        for h in heads:
            qofs[h] = o
            phys2h.append(h)
            o += HD
        dq += wid
        blocks.append(["k", (2 * i, 2 * i + 1), 256, 0, QW + i * 256])
    # interleave order: q,k alternating already by construction above
    order = []
    for i in range(8):
        order.append(blocks[2 * i])
        order.append(blocks[2 * i + 1])
    for i in range(8):
        order.append(["v", (2 * i, 2 * i + 1), 256, 0, i * 256])
    # W' column offsets in block order (host packs W' accordingly)
    ofs = 0
    for blk in order:
        blk[3] = ofs
        ofs += blk[2]
    assert ofs == WCOLS
    return order, phys2h, qofs


BLOCKS, PHYS2H, QOFS = _build_blocks()
H2ROW = {h: r for r, h in enumerate(PHYS2H)}
NB = len(BLOCKS)  # 24
QBLK = {}  # head -> block its q chunk lands
KBLK = {}
VBLK = {}
for bi, (kind, heads, wid, wofs, dofs) in enumerate(BLOCKS):
    for hh in heads:
        if kind == "q":
            QBLK[hh] = bi
        elif kind == "k":
            KBLK[hh] = bi
        else:
            VBLK[hh] = bi

# ---------------- op cost menu (ns, cost-model calibrated) ----------------
CC = {
    "sD": 194.0,  # DVE fused STT+accum per pair
    "multD": lambda n: 128 * n * 0.521 + 80.0,  # DVE TT bcast mult (2x)
    "multP": lambda n: 128 * n * 1.984 + 95.0,  # Pool TT bcast mult
    "redD": lambda n: 128 * n * 1.042 + 85.0,  # DVE segmented reduce
    "redA": 479.0,  # Act per-pair reduce (incl. accum read)
    "mulP": 273.0,
    "mulA": 292.0,
    "addD": lambda n: 128 * n * 0.521 + 64.0,  # DVE 2x TT add over n*HD
    "addP": 349.0,
    "stt": 194.0,
    "cpD": 373.0,
    "cpA": 398.0,
    "cpP": 435.0,
    "cpD128": 252.0,
    "cpA128": 292.0,
    "cpP128": 270.0,
    "exp": 398.0,
    "sm": 700.0,  # DVE softmax chain per tile
    "memset": 310.0,
    "init": 273.0,  # Pool ts_mul -> acc
    "initA": 292.0,
    "head0": 292.0,
    "dmaseq": 667.0,  # Act SEQ time per dma_start
}

D, A, PL = "D", "A", "P"

_FILL = float(os.environ.get("K2_FILL", "0.93"))
_RUNW_AV = int(os.environ.get("K2_RUNW_AV", "4"))
_RUNW_S = int(os.environ.get("K2_RUNW_S", "8"))
_SCORE_DEADLINE = int(os.environ.get("K2_SCORE_DL", "16"))
_WSIZES = os.environ.get("K2_WSIZES", "3,3,3,3,2,2")
_XP2 = int(os.environ.get("K2_XP", "8"))
_POST_ACT = float(os.environ.get("K2_POST_ACT", "1.0"))
_OUTSPLIT = int(os.environ.get("K2_OUTSPLIT", "0"))
_LAST_STT = int(os.environ.get("K2_LAST_STT", "1"))
_AVPB = int(os.environ.get("K2_AVPB", "24"))
_XQ = os.environ.get("K2_XQ", "A")
_WP2 = int(os.environ.get("K2_WP", "2"))
_PRODP2 = int(os.environ.get("K2_PRODP", "10"))
_OUTQ = os.environ.get("K2_OUTQ", "S")


def _parse_wsizes(nt):
    ws = [int(x) for x in _WSIZES.split(",")]
    if sum(ws) != nt:
        # fall back: windows of TG
        ws = [TG] * (nt // TG)
        r = nt - sum(ws)
        if r:
            ws.append(r)
    return ws
_PE_NS = 0.41667


def _blockdur(width, tiles):
    return tiles * DC * width * _PE_NS


# ---------------- static scheduler ----------------


class Ledger:
    def __init__(self, nwin, tiles_per_win, npost):
        self.nwin = nwin
        self.tpw = tiles_per_win
        self.caps = []
        self.durs = []
        for w in range(nwin):
            for b in range(NB):
                d = _blockdur(BLOCKS[b][2], tiles_per_win[w])
                self.durs.append(d)
                self.caps.append({D: 0.0, A: 0.0, PL: 0.0})
        self.post0 = len(self.caps)
        for _ in range(npost):
            self.durs.append(3000.0)
            self.caps.append({D: 0.0, A: 0.0, PL: 0.0})
        self.n = len(self.caps)

    def gb(self, w, b):
        return w * NB + b

    def room(self, eng, g):
        return self.durs[g] * _FILL - self.caps[g][eng]

    def load(self, eng, g, dur):
        self.caps[g][eng] += dur

    def fill_of(self, eng, g):
        if g >= self.post0 and eng == A:
            return _FILL * _POST_ACT
        return _FILL

    def place(self, opts, ready, deadline=None):
        """opts: list of (engine, dur). Returns (engine, gblock)."""
        ready = max(0, min(ready, self.n - 1))
        best = None
        for g in range(ready, self.n):
            cands = [(self.caps[g][e] + du - self.durs[g] * self.fill_of(e, g),
                      e, du) for e, du in opts]
            cands.sort()
            slack, e, du = cands[0]
            if slack <= 0.0:
                self.load(e, g, du)
                return e, g
            if best is None or slack < best[0]:
                best = (slack, e, du, g)
            if deadline is not None and g >= deadline:
                break
        slack, e, du, g = best
        self.load(e, g, du)
        return e, g


def plan(nwin, tiles_per_win, npost=40):
    """Returns sched: dict gblock -> list of op dicts (in placement order),
    plus per-(w,t) out-DMA block."""
    led = Ledger(nwin, tiles_per_win, npost)
    sched = {}

    def put(w, op, g):
        op["w"] = w
        sched.setdefault(g, []).append(op)

    copy_eng = {}
    # pass 1: copies (fixed block, balance engines)
    for w in range(nwin):
        T = tiles_per_win[w]
        for b in range(NB):
            wid = BLOCKS[b][2]
            for t in range(T):
                if wid == 128:
                    opts = [(A, CC["cpA128"]), (PL, CC["cpP128"]),
                            (D, CC["cpD128"])]
                else:
                    opts = [(A, CC["cpA"]), (PL, CC["cpP"]), (D, CC["cpD"])]
                g = led.gb(w, b)
                # prefer Act/Pool; D only if both are past fill
                e, du = opts[0]
                for e2, du2 in opts:
                    if led.caps[g][e2] + du2 <= led.durs[g] * _FILL:
                        e, du = e2, du2
                        break
                led.load(e, g, du)
                copy_eng[(w, b, t)] = e
        # x-prefetch + memset bookkeeping
        if w + 1 < nwin:
            for i in range(tiles_per_win[w + 1]):
                led.load(A, led.gb(w, min(i, NB - 1)), CC["dmaseq"])
        for t in range(T):
            g = led.gb(w, 0)
            e, g2 = led.place([(PL, CC["memset"]), (D, CC["memset"])],
                              g, deadline=g)
            put(w, {"k": "memset", "t": t, "e": e}, g2)

    # pass 2: scores
    last_score = {}
    for w in range(nwin):
        T = tiles_per_win[w]
        units = []
        for g in range(HEADS):
            kb = KBLK[g]
            pre_rows = sorted(H2ROW[h] for h in range(max(g, 1), HEADS)
                              if QBLK[h] < kb)
            # split into physically-contiguous row runs (h >= g can cut
            # into the middle of a q chunk for odd g)
            run = []
            for r in pre_rows + [None]:
                if run and (r is None or r != run[-1] + 1):
                    units.append({"g": g,
                                  "rows": [PHYS2H[x] for x in run],
                                  "ready": kb})
                    run = []
                if r is not None:
                    run.append(r)
            rest = sorted(h for h in range(max(g, 1), HEADS) if QBLK[h] > kb)
            i = 0
            while i < len(rest):
                h = rest[i]
                pair = [h]
                if i + 1 < len(rest) and QBLK[rest[i + 1]] == QBLK[h]:
                    pair.append(rest[i + 1])
                units.append({"g": g, "rows": pair, "ready": QBLK[h]})
                i += len(pair)
        units.sort(key=lambda u: u["ready"])
        for t in range(T):
            for u in units:
                rows = u["rows"]
                gq = led.gb(w, u["ready"])
                dl = led.gb(w, min(_SCORE_DEADLINE + t, NB - 1))
                if len(rows) <= 2:
                    for h in rows:
                        e, g2 = led.place([(D, CC["sD"])], gq, deadline=dl)
                        put(w, {"k": "sD", "t": t, "h": h, "g": u["g"]}, g2)
                        last_score[(w, t)] = max(
                            last_score.get((w, t), 0), g2)
                else:
                    i = 0
                    while i < len(rows):
                        seg = rows[i:i + _RUNW_S]
                        n = len(seg)
                        if n < 3:
                            for h in seg:
                                e, g2 = led.place([(D, CC["sD"])], gq,
                                                  deadline=dl)
                                put(w, {"k": "sD", "t": t, "h": h,
                                        "g": u["g"]}, g2)
                                last_score[(w, t)] = max(
                                    last_score.get((w, t), 0), g2)
                        else:
                            e, g2 = led.place(
                                [(D, CC["multD"](n)), (PL, CC["multP"](n))],
                                gq, deadline=dl)
                            e3, g3 = led.place([(D, CC["redD"](n))], g2,
                                               deadline=dl)
                            put(w, {"k": "mult", "t": t, "rows": seg,
                                    "g": u["g"], "e": e, "uid": (w, t, u["g"], i)}, g2)
                            put(w, {"k": "red", "t": t, "rows": seg,
                                    "g": u["g"], "uid": (w, t, u["g"], i)}, g3)
                            last_score[(w, t)] = max(
                                last_score.get((w, t), 0), g3)
                        i += n

    # pass 3: exp + softmax per tile
    sm_done = {}
    for w in range(nwin):
        wend = led.gb(w + 1, NB - 1) if w + 1 < nwin else led.n - 1
        for t in range(tiles_per_win[w]):
            r = last_score[(w, t)] + 1
            e, g1 = led.place([(A, CC["exp"])], r, deadline=wend)
            e, g2 = led.place([(D, CC["sm"])], g1, deadline=wend)
            put(w, {"k": "exp", "t": t}, g1)
            put(w, {"k": "softmax", "t": t}, g2)
            sm_done[(w, t)] = g2

    # pass 4: AV (inits column 0, then batched columns 1..15), chains per row
    outblk = {}
    for w in range(nwin):
        T = tiles_per_win[w]
        wend = led.gb(w + 1, NB - 1) if w + 1 < nwin else led.n - 1
        chain = {}
        for t in range(T):
            base = max(sm_done[(w, t)] + 1, led.gb(w, VBLK[0]) + 1)
            e, g0 = led.place([(A, CC["head0"])], base, deadline=wend)
            put(w, {"k": "head0", "t": t}, g0)
            done = g0
            for h in range(1, HEADS):
                e, gi = led.place([(PL, CC["init"]), (A, CC["initA"])], base,
                                  deadline=wend)
                put(w, {"k": "init", "t": t, "h": h, "e": e}, gi)
                chain[(t, h)] = gi
                done = max(done, gi)
            for g in range(1, HEADS):
                vb = led.gb(w, VBLK[g]) + (0 if VBLK[g] == NB - 1 else 1)
                rdy0 = max(sm_done[(w, t)] + 1, vb)
                h = g
                lastwin = (w == nwin - 1) and _LAST_STT
                while h < HEADS:
                    seg = list(range(h, min(h + (1 if lastwin else _RUNW_AV),
                                            HEADS)))
                    n = len(seg)
                    rdy = max([rdy0] + [chain[(t, hh)] for hh in seg])
                    if n == 1 and lastwin:
                        e, gm = led.place(
                            [(D, CC["stt"]), (A, CC["mulA"] + 40.0)], rdy,
                            deadline=wend)
                        if e == D:
                            put(w, {"k": "stt", "t": t, "h": seg[0],
                                    "g": g}, gm)
                        else:
                            uid = (w, t, g, seg[0])
                            led.load(PL, gm, CC["addP"])
                            put(w, {"k": "avmul", "t": t, "h": seg[0],
                                    "g": g, "e": A, "uid": uid, "n": 1,
                                    "j": 0}, gm)
                            put(w, {"k": "avadd_p", "t": t, "rows": seg,
                                    "g": g, "uid": uid}, gm)
                        ga = gm
                    elif n == 1:
                        e, gm = led.place([(D, CC["stt"])], rdy,
                                          deadline=wend)
                        put(w, {"k": "stt", "t": t, "h": seg[0], "g": g}, gm)
                        ga = gm
                    else:
                        muls = []
                        gm = rdy
                        for j, hh in enumerate(seg):
                            e, gmu = led.place(
                                [(PL, CC["mulP"]), (A, CC["mulA"])], rdy,
                                deadline=wend)
                            muls.append((hh, e, gmu))
                            gm = max(gm, gmu)
                        e, ga = led.place([(D, CC["addD"](n))], gm,
                                          deadline=wend)
                        uid = (w, t, g, seg[0])
                        for hh, e2, gmu in muls:
                            put(w, {"k": "avmul", "t": t, "h": hh, "g": g,
                                    "e": e2, "uid": uid, "n": n,
                                    "j": seg.index(hh)}, gmu)
                        put(w, {"k": "avadd", "t": t, "rows": seg, "g": g,
                                "uid": uid}, ga)
                    for hh in seg:
                        chain[(t, hh)] = ga
                    done = max(done, ga)
                    h += n
            g_out = min(done + 1, wend)
            if _OUTSPLIT:
                done_lo = max([g0] + [chain[(t, hh)] for hh in range(1, 8)])
                g_lo = min(done_lo + 1, wend)
                led.load(A, g_lo, CC["dmaseq"])
                put(w, {"k": "out_lo", "t": t}, g_lo)
                led.load(A, g_out, CC["dmaseq"])
                put(w, {"k": "out_hi", "t": t}, g_out)
            else:
                led.load(A, g_out, CC["dmaseq"])
                put(w, {"k": "out_hi", "t": t}, g_out)
            outblk[(w, t)] = g_out

    return led, sched, copy_eng


# ---------------- emission ----------------


def _emit_op(nc, op, ctx, aux):
    k = op["k"]
    w = op["w"]
    c = ctx[w]
    t = op.get("t")
    trD, trA = aux["trD"], aux["trA"]
    avp, prodp = aux["avp"], aux["prodp"]
    if k == "memset":
        eng = nc.gpsimd if op["e"] == PL else nc.vector
        eng.memset(c["s"][t], -30000.0)
    elif k == "sD":
        h, g = op["h"], op["g"]
        qt = c["qk"][t]
        aux["ti"] += 1
        nc.vector.scalar_tensor_tensor(
            out=trD[aux["ti"] % len(trD)],
            in0=qt[:, QOFS[h]:QOFS[h] + HD],
            scalar=1.0,
            in1=qt[:, QW + g * HD:QW + (g + 1) * HD],
            op0=ALU.bypass, op1=ALU.mult,
            accum_out=c["s"][t][:, H2ROW[h] * HEADS + g:
                                H2ROW[h] * HEADS + g + 1],
        )
    elif k == "mult":
        rows, g = op["rows"], op["g"]
        qt = c["qk"][t]
        n = len(rows)
        r0 = H2ROW[rows[0]]
        prod = prodp.tile([P, _RUNW_S * HD], F16, name="prod", tag="prod")
        aux["prods"][op["uid"]] = prod
        kg = qt[:, QW + g * HD:QW + (g + 1) * HD]
        eng = nc.vector if op["e"] == D else nc.gpsimd
        eng.tensor_tensor(
            out=prod[:, 0:n * HD].rearrange("p (n d) -> p n d", d=HD),
            in0=qt[:, r0 * HD:(r0 + n) * HD].rearrange(
                "p (n d) -> p n d", d=HD),
            in1=kg[:, None, :].to_broadcast((P, n, HD)),
            op=ALU.mult,
        )
    elif k == "red":
        rows, g = op["rows"], op["g"]
        n = len(rows)
        r0 = H2ROW[rows[0]]
        prod = aux["prods"].pop(op["uid"])
        s3 = c["s"][t].rearrange("p (r g) -> p r g", g=HEADS)
        nc.vector.tensor_reduce(
            s3[:, r0:r0 + n, g],
            prod[:, 0:n * HD].rearrange("p (n d) -> p n d", d=HD),
            axis=mybir.AxisListType.X, op=ALU.add,
        )
    elif k == "exp":
        if aux.get("sdbg") is not None and w == 0 and t == 0:
            nc.sync.dma_start(aux["sdbg"], c["s"][t])
        nc.scalar.activation(c["s"][t], c["s"][t], ACTF.Exp)
    elif k == "softmax":
        s3 = c["s"][t].rearrange("p (r g) -> p r g", g=HEADS)
        sumE = c["sm"][t][:, 0:NROW]
        recip = c["sm"][t][:, NROW:2 * NROW]
        nc.vector.tensor_reduce(sumE, s3, axis=mybir.AxisListType.X,
                                op=ALU.add)
        nc.vector.reciprocal(recip, sumE)
        nc.vector.tensor_tensor(
            out=s3, in0=s3,
            in1=recip[:, :, None].to_broadcast((P, NROW, HEADS)),
            op=ALU.mult,
        )
    elif k == "head0":
        nc.scalar.copy(c["acc"][t][:, 0:HD], c["v"][t][:, 0:HD])
    elif k == "init":
        h = op["h"]
        pcol = c["s"][t][:, H2ROW[h] * HEADS:H2ROW[h] * HEADS + 1]
        ah = c["acc"][t][:, h * HD:(h + 1) * HD]
        if op["e"] == PL:
            nc.gpsimd.tensor_scalar_mul(ah, c["v"][t][:, 0:HD], pcol)
        else:
            nc.scalar.activation(ah, c["v"][t][:, 0:HD], ACTF.Copy,
                                 scale=pcol)
    elif k == "stt":
        h, g = op["h"], op["g"]
        pcol = c["s"][t][:, H2ROW[h] * HEADS + g:H2ROW[h] * HEADS + g + 1]
        ah = c["acc"][t][:, h * HD:(h + 1) * HD]
        nc.vector.scalar_tensor_tensor(
            out=ah, in0=c["v"][t][:, g * HD:(g + 1) * HD], scalar=pcol,
            in1=ah, op0=ALU.mult, op1=ALU.add)
    elif k == "avmul":
        h, g, n, j = op["h"], op["g"], op["n"], op["j"]
        uid = op["uid"]
        if uid not in aux["avtmp"]:
            aux["avtmp"][uid] = avp.tile([P, _RUNW_AV * HD], F16,
                                         name="avtmp", tag="avtmp")
        tmp = aux["avtmp"][uid]
        pcol = c["s"][t][:, H2ROW[h] * HEADS + g:H2ROW[h] * HEADS + g + 1]
        vs = c["v"][t][:, g * HD:(g + 1) * HD]
        if op["e"] == PL:
            nc.gpsimd.tensor_scalar_mul(tmp[:, j * HD:(j + 1) * HD], vs, pcol)
        else:
            nc.scalar.activation(tmp[:, j * HD:(j + 1) * HD], vs, ACTF.Copy,
                                 scale=pcol)
    elif k in ("avadd", "avadd_p", "avadd_a"):
        rows = op["rows"]
        n = len(rows)
        h0 = rows[0]
        tmp = aux["avtmp"].pop(op["uid"])
        ar = c["acc"][t][:, h0 * HD:(h0 + n) * HD]
        eng = {"avadd": nc.vector, "avadd_p": nc.gpsimd,
               "avadd_a": None}[k]
        if k == "avadd_a":
            nc.scalar.activation(ar, tmp[:, 0:n * HD], ACTF.Copy,
                                 bias=ar)
        else:
            eng.tensor_tensor(out=ar, in0=ar, in1=tmp[:, 0:n * HD],
                              op=ALU.add)
    elif k in ("out_lo", "out_hi"):
        ti = c["tile0"] + t
        eng = {"A": nc.scalar, "S": nc.sync, "P": nc.gpsimd}[_OUTQ]
        half = DIMS // 2
        if k == "out_hi" and not _OUTSPLIT:
            eng.dma_start(aux["out"][ti * P:(ti + 1) * P, :], c["acc"][t])
        elif k == "out_lo":
            eng.dma_start(aux["out"][ti * P:(ti + 1) * P, 0:half],
                          c["acc"][t][:, 0:half])
        else:
            eng.dma_start(aux["out"][ti * P:(ti + 1) * P, half:DIMS],
                          c["acc"][t][:, half:DIMS])
    else:
        raise ValueError(k)


def _body(tc, xt, wt, out, nt, reps=1, wsizes=None, sdbg=None):
    nc = tc.nc
    if wsizes is None:
        wsizes = _parse_wsizes(nt)
    assert sum(wsizes) == nt, (wsizes, nt)
    nwin_real = len(wsizes)
    nwin = nwin_real * reps
    tiles_per_win = list(wsizes) * reps
    tile0s = []
    o = 0
    for sz in wsizes:
        tile0s.append(o)
        o += sz
    led, sched, copy_eng = plan(nwin, tiles_per_win)
    dsub = DC // WSUB
    with (
        tc.tile_pool(name="xp", bufs=_XP2) as xp,
        tc.tile_pool(name="wp", bufs=_WP2) as wp,
        tc.tile_pool(name="qkp", bufs=TG + 2) as qkp,
        tc.tile_pool(name="vp", bufs=2 * TG) as vp,
        tc.tile_pool(name="psum", bufs=8, space="PSUM") as pp,
        tc.tile_pool(name="sp", bufs=2 * TG) as sp,
        tc.tile_pool(name="smallp", bufs=2 * TG) as smallp,
        tc.tile_pool(name="accp", bufs=TG + 2) as accp,
        tc.tile_pool(name="avp", bufs=_AVPB) as avp,
        tc.tile_pool(name="prodp", bufs=_PRODP2) as prodp,
        tc.tile_pool(name="trashp", bufs=1) as trashp,
    ):
        trD = [trashp.tile([P, HD], F16, name="trash_d", tag=f"trash_d{i}")
               for i in range(8)]
        trA = [trashp.tile([P, HD], F16, name="trash_a", tag=f"trash_a{i}")
               for i in range(8)]
        aux = {"trD": trD, "trA": trA, "avp": avp, "prodp": prodp,
               "out": out, "ti": 0, "prods": {}, "avtmp": {},
               "copy_eng": copy_eng, "sdbg": sdbg}
        ctx = {}
        xeng = {"A": nc.scalar, "S": nc.sync, "P": nc.gpsimd}[_XQ]
        xg_next = [xp.tile([P, DC * P], F16, name="xg", tag="xg")
                   for _ in range(wsizes[0])]
        for t in range(wsizes[0]):
            xeng.dma_start(xg_next[t], xt[t])
        for w in range(nwin):
            gi = w % nwin_real
            T = tiles_per_win[w]
            xg = xg_next
            if w + 1 < nwin:
                gn = (w + 1) % nwin_real
                xg_next = [xp.tile([P, DC * P], F16, name="xg", tag="xg")
                           for _ in range(tiles_per_win[w + 1])]
                for t in range(tiles_per_win[w + 1]):
                    xeng.dma_start(xg_next[t], xt[tile0s[gn] + t])
            ctx[w] = {
                "tile0": tile0s[gi],
                "qk": [qkp.tile([P, QW + KW], F16, name="qk", tag="qk")
                       for _ in range(T)],
                "v": [vp.tile([P, DIMS], F16, name="v", tag="v")
                      for _ in range(T)],
                "s": [sp.tile([P, NROW * HEADS], F32, name="s", tag="s")
                      for _ in range(T)],
                "sm": [smallp.tile([P, 2 * NROW], F32, name="sm", tag="sm")
                       for _ in range(T)],
                "acc": [accp.tile([P, DIMS], F16, name="acc", tag="acc")
                        for _ in range(T)],
            }
            if w - 2 in ctx:
                del ctx[w - 2]
            for b in range(NB):
                kind, heads, wid, wofs, dofs = BLOCKS[b]
                wtile = wp.tile([P, DC, 256], F16, name="w", tag="w")
                for sub in range(WSUB):
                    nc.sync.dma_start(
                        wtile[:, sub * dsub:(sub + 1) * dsub, 0:wid],
                        wt[:, sub * dsub:(sub + 1) * dsub, wofs:wofs + wid],
                    )
                for t in range(T):
                    ps = pp.tile([P, 256], F32, name="ps", tag="ps")
                    for d in range(DC):
                        nc.tensor.matmul(
                            ps[:, 0:wid],
                            lhsT=xg[t][:, d * P:(d + 1) * P],
                            rhs=wtile[:, d, 0:wid],
                            start=(d == 0),
                            stop=(d == DC - 1),
                        )
                    if kind == "v":
                        dst = ctx[w]["v"][t][:, dofs:dofs + wid]
                    else:
                        dst = ctx[w]["qk"][t][:, dofs:dofs + wid]
                    # copies always emitted on planner-chosen engine
                    e = aux.get("copy_eng", {}).get((w, b, t), A)
                    if e == D:
                        nc.vector.tensor_copy(dst, ps[:, 0:wid])
                    elif e == PL:
                        nc.gpsimd.tensor_copy(dst, ps[:, 0:wid])
                    else:
                        nc.scalar.copy(dst, ps[:, 0:wid])
                for op in sched.get(led.gb(w, b), []):
                    _emit_op(nc, op, ctx, aux)
        for g in range(nwin * NB, led.n):
            for op in sched.get(g, []):
                _emit_op(nc, op, ctx, aux)


def build(tpc=TPC, reps=1):
    import concourse.bacc as bacc

    nc = bacc.Bacc(None, target_bir_lowering=False, debug=False,
                   enable_asserts=True, num_devices=NCORES)
    nt = tpc // P
    xt = nc.dram_tensor("xt", [nt, P, DC * P], F16, kind="ExternalInput").ap()
    wt = nc.dram_tensor("wt", [P, DC, WCOLS], F16, kind="ExternalInput").ap()
    out = nc.dram_tensor("out", [tpc, DIMS], F16, kind="ExternalOutput").ap()
    sdbg = None
    if int(os.environ.get("K2_DBG_S", "0")):
        sdbg = nc.dram_tensor("sdbg", [P, NROW * HEADS], F32,
                              kind="ExternalOutput").ap()
    with tile.TileContext(nc) as tc:
        _body(tc, xt, wt, out, nt, reps=reps, sdbg=sdbg)
    nc.compile()
    return nc


def prep_inputs(input_seq_embs, Wq, Wk, Wv, tpc=TPC, ncores=NCORES):
    x = np.asarray(input_seq_embs, dtype=np.float32).reshape(TOK, DIMS)
    Wq = np.asarray(Wq)
    Wk = np.asarray(Wk)
    Wv = np.asarray(Wv)
    rows = []
    for kind, heads, wid, wofs, dofs in BLOCKS:
        Wsrc = {"q": Wq, "k": Wk, "v": Wv}[kind]
        for h in heads:
            rows.append(Wsrc[h * HD:(h + 1) * HD])
    wall = np.concatenate(rows, axis=0).astype(np.float32)  # [WCOLS, DIMS]
    wtile = np.ascontiguousarray(
        wall.T.reshape(DC, P, WCOLS).transpose(1, 0, 2)
    ).astype(np.float16)
    in_maps = []
    nt = tpc // P
    for c in range(ncores):
        xs = x[c * tpc:(c + 1) * tpc]
        xtile = (
            xs.reshape(nt, P, DC, P).transpose(0, 3, 2, 1).astype(np.float16)
        ).reshape(nt, P, DC * P)
        in_maps.append({"xt": np.ascontiguousarray(xtile), "wt": wtile})
    return in_maps


_cached = {}


def _get_nc():
    if "nc" not in _cached:
        _cached["nc"] = build()
    return _cached["nc"]


def kernel_with_results(**inputs):
    from concourse import bass_utils

    nc = _get_nc()
    in_maps = prep_inputs(**inputs)
    res = bass_utils.run_bass_kernel_spmd(
        nc, in_maps, core_ids=list(range(NCORES)), trace=False)
    outs = [r["out"] for r in res.results]
    full = (np.concatenate(outs, axis=0).astype(np.float32)
            .reshape(B, L, DIMS))
    return full, res


def kernel(**inputs):
    return kernel_with_results(**inputs)[0]


# revision 9
# speedup vs baseline: 3083.3250x; 1.0049x over previous
"""Trainium2 Bass kernel v2 for nn_MultiHeadAttention_47115791237226.

TimelineSim (TRN2 cost model): 707,349 ns vs 733,763 ns for v1 (-3.6%).
Verified on the 8-core emulated device: rel err 1.25e-3 (gate 2e-2).

Differences vs v1 (kernel_baseline.py):
 - q head 0 is never computed (row h=0 of A is fully masked except A[0,0],
   whose softmax is 1 regardless -> out head 0 = v head 0). W' has 6016
   output columns; the q1 chunk is 128 wide -> 2% less PE work.
 - qk chunk order interleaves Q (descending head pairs) with K (ascending):
   [Q7 K0 Q6 K1 ... Q1 K6 Q0' K7] then V0..V7. Score pair (h,g) becomes
   ready at max(qblk(h), kblk(g)) -> score work spreads over blocks 1..15
   instead of 8..15, and heavy rows (high h) are ready earliest.
 - a global static scheduler places every attention op (score units, exp,
   softmax, AV units, copies, out-DMA) into (window, block) slots against
   per-engine capacity ledgers. AV for group g starts inside window g's
   v-region ("early AV"); what doesn't fit spills into the next window's
   early blocks; only the last window's small spill runs post-matmul.
 - AV adds are batched: n pair-muls (Act/Pool) write one tmp [P,n*128],
   then a single DVE 2x TT add updates acc[:, h0*128:(h0+n)*128].
 - score units: 2-row cohorts -> fused DVE STT+accum (sD); >=3-row runs ->
   one broadcast mult (DVE or Pool) + one DVE segmented tensor_reduce.

s-tile rows are in PHYSICAL q order (phys2h = 14,15,12,13,...,2,3,1);
softmax is row-order independent; AV reads map h -> row.

kernel() accepts the FULL unsharded inputs and returns the FULL output.
"""

import os
import sys

import numpy as np

sys.path.insert(0, "/opt/trn_rl_repo")

import concourse.mybir as mybir  # noqa: E402
import concourse.tile as tile  # noqa: E402

DIMS = 2048
HEADS = 16
HD = DIMS // HEADS  # 128
B, L = 4, 4096
TOK = B * L
NCORES = 8
TPC = TOK // NCORES  # 2048
P = 128
DC = DIMS // P  # 16
TG = 4  # tiles per W window
WSUB = int(os.environ.get("K2_WSUB", "4"))  # sub-DMAs per W chunk
NROW = 15  # score rows (heads 1..15)
QW = NROW * HD  # 1920 q-region width
KW = DIMS  # 2048
WCOLS = QW + 2 * DIMS  # 6016

F16 = mybir.dt.float16
F32 = mybir.dt.float32
ALU = mybir.AluOpType
ACTF = mybir.ActivationFunctionType

# ---------------- chunk / block tables ----------------
# qk blocks 0..15: even b -> Q chunk (descending), odd b -> K chunk (asc).
# blocks 16..23: V chunks ascending.
# Each entry: (kind, heads, width, wofs, dofs) where wofs = column offset in
# the host-packed W' (block-major), dofs = dest offset in qk/v SBUF region.


def _build_blocks():
    qchunks = []  # descending: Q7=(14,15) ... Q1=(2,3), Q0'=(1,)
    for c in range(7, 0, -1):
        qchunks.append(((2 * c, 2 * c + 1), 256))
    qchunks.append(((1,), 128))
    blocks = []
    phys2h = []
    qofs = {}
    dq = 0
    for i in range(8):
        heads, wid = qchunks[i]
        blocks.append(["q", heads, wid, 0, dq])
        for h in heads:
            qofs[h] = dq + (len(qofs) - len(qofs)) * 0  # placeholder
        o = dq
        for h in heads:
            qofs[h] = o
            phys2h.append(h)
            o += HD
        dq += wid
        blocks.append(["k", (2 * i, 2 * i + 1), 256, 0, QW + i * 256])
    # interleave order: q,k alternating already by construction above
    order = []
    for i in range(8):
        order.append(blocks[2 * i])
        order.append(blocks[2 * i + 1])
    for i in range(8):
        order.append(["v", (2 * i, 2 * i + 1), 256, 0, i * 256])
    # W' column offsets in block order (host packs W' accordingly)
    ofs = 0
    for blk in order:
        blk[3] = ofs
        ofs += blk[2]
    assert ofs == WCOLS
    return order, phys2h, qofs


BLOCKS, PHYS2H, QOFS = _build_blocks()
H2ROW = {h: r for r, h in enumerate(PHYS2H)}
NB = len(BLOCKS)  # 24
QBLK = {}  # head -> block its q chunk lands
KBLK = {}
VBLK = {}
for bi, (kind, heads, wid, wofs, dofs) in enumerate(BLOCKS):
    for hh in heads:
        if kind == "q":
            QBLK[hh] = bi
        elif kind == "k":
            KBLK[hh] = bi
        else:
            VBLK[hh] = bi

# ---------------- op cost menu (ns, cost-model calibrated) ----------------
CC = {
    "sD": 194.0,  # DVE fused STT+accum per pair
    "multD": lambda n: 128 * n * 0.521 + 80.0,  # DVE TT bcast mult (2x)
    "multP": lambda n: 128 * n * 1.984 + 95.0,  # Pool TT bcast mult
    "redD": lambda n: 128 * n * 1.042 + 85.0,  # DVE segmented reduce
    "redA": 479.0,  # Act per-pair reduce (incl. accum read)
    "mulP": 273.0,
    "mulA": 292.0,
    "addD": lambda n: 128 * n * 0.521 + 64.0,  # DVE 2x TT add over n*HD
    "addP": 349.0,
    "stt": 194.0,
    "cpD": 373.0,
    "cpA": 398.0,
    "cpP": 435.0,
    "cpD128": 252.0,
    "cpA128": 292.0,
    "cpP128": 270.0,
    "exp": 398.0,
    "sm": 700.0,  # DVE softmax chain per tile
    "memset": 310.0,
    "init": 273.0,  # Pool ts_mul -> acc
    "initA": 292.0,
    "head0": 292.0,
    "dmaseq": 667.0,  # Act SEQ time per dma_start
}

D, A, PL = "D", "A", "P"

_FILL = float(os.environ.get("K2_FILL", "0.93"))
_RUNW_AV = int(os.environ.get("K2_RUNW_AV", "4"))
_RUNW_S = int(os.environ.get("K2_RUNW_S", "8"))
_SCORE_DEADLINE = int(os.environ.get("K2_SCORE_DL", "16"))
_WSIZES = os.environ.get("K2_WSIZES", "3,3,3,3,2,2")
_XP2 = int(os.environ.get("K2_XP", "8"))
_POST_ACT = float(os.environ.get("K2_POST_ACT", "1.0"))
_OUTSPLIT = int(os.environ.get("K2_OUTSPLIT", "0"))
_LAST_STT = int(os.environ.get("K2_LAST_STT", "1"))
_AVPB = int(os.environ.get("K2_AVPB", "24"))
_OUT_DEFER = int(os.environ.get("K2_OUT_DEFER", "8"))
_POST_DUR = float(os.environ.get("K2_POST_DUR", "6000"))
_XISSUE = int(os.environ.get("K2_XISSUE", "0"))
_XQ = os.environ.get("K2_XQ", "A")
_WP2 = int(os.environ.get("K2_WP", "2"))
_PRODP2 = int(os.environ.get("K2_PRODP", "10"))
_OUTQ = os.environ.get("K2_OUTQ", "S")


def _parse_wsizes(nt):
    ws = [int(x) for x in _WSIZES.split(",")]
    if sum(ws) != nt:
        # fall back: windows of TG
        ws = [TG] * (nt // TG)
        r = nt - sum(ws)
        if r:
            ws.append(r)
    return ws
_PE_NS = 0.41667


def _blockdur(width, tiles):
    return tiles * DC * width * _PE_NS


# ---------------- static scheduler ----------------


class Ledger:
    def __init__(self, nwin, tiles_per_win, npost):
        self.nwin = nwin
        self.tpw = tiles_per_win
        self.caps = []
        self.durs = []
        for w in range(nwin):
            for b in range(NB):
                d = _blockdur(BLOCKS[b][2], tiles_per_win[w])
                self.durs.append(d)
                self.caps.append({D: 0.0, A: 0.0, PL: 0.0})
        self.post0 = len(self.caps)
        for _ in range(npost):
            self.durs.append(_POST_DUR)
            self.caps.append({D: 0.0, A: 0.0, PL: 0.0})
        self.n = len(self.caps)

    def gb(self, w, b):
        return w * NB + b

    def room(self, eng, g):
        return self.durs[g] * _FILL - self.caps[g][eng]

    def load(self, eng, g, dur):
        self.caps[g][eng] += dur

    def fill_of(self, eng, g):
        if g >= self.post0 and eng == A:
            return _FILL * _POST_ACT
        return _FILL

    def place(self, opts, ready, deadline=None):
        """opts: list of (engine, dur). Returns (engine, gblock)."""
        ready = max(0, min(ready, self.n - 1))
        best = None
        for g in range(ready, self.n):
            cands = [(self.caps[g][e] + du - self.durs[g] * self.fill_of(e, g),
                      e, du) for e, du in opts]
            cands.sort()
            slack, e, du = cands[0]
            if slack <= 0.0:
                self.load(e, g, du)
                return e, g
            if best is None or slack < best[0]:
                best = (slack, e, du, g)
            if deadline is not None and g >= deadline:
                break
        slack, e, du, g = best
        self.load(e, g, du)
        return e, g


def plan(nwin, tiles_per_win, npost=max(40, int(120000 / _POST_DUR))):
    """Returns sched: dict gblock -> list of op dicts (in placement order),
    plus per-(w,t) out-DMA block."""
    led = Ledger(nwin, tiles_per_win, npost)
    sched = {}

    def put(w, op, g):
        op["w"] = w
        sched.setdefault(g, []).append(op)

    copy_eng = {}
    # pass 1: copies (fixed block, balance engines)
    for w in range(nwin):
        T = tiles_per_win[w]
        for b in range(NB):
            wid = BLOCKS[b][2]
            for t in range(T):
                if wid == 128:
                    opts = [(A, CC["cpA128"]), (PL, CC["cpP128"]),
                            (D, CC["cpD128"])]
                else:
                    opts = [(A, CC["cpA"]), (PL, CC["cpP"]), (D, CC["cpD"])]
                g = led.gb(w, b)
                # prefer Act/Pool; D only if both are past fill
                e, du = opts[0]
                for e2, du2 in opts:
                    if led.caps[g][e2] + du2 <= led.durs[g] * _FILL:
                        e, du = e2, du2
                        break
                led.load(e, g, du)
                copy_eng[(w, b, t)] = e
        # x-prefetch + memset bookkeeping
        if w + 1 < nwin:
            for i in range(tiles_per_win[w + 1]):
                led.load(A, led.gb(w, min(i, NB - 1)), CC["dmaseq"])
        for t in range(T):
            g = led.gb(w, 0)
            e, g2 = led.place([(PL, CC["memset"]), (D, CC["memset"])],
                              g, deadline=g)
            put(w, {"k": "memset", "t": t, "e": e}, g2)

    # pass 2: scores
    last_score = {}
    for w in range(nwin):
        T = tiles_per_win[w]
        units = []
        for g in range(HEADS):
            kb = KBLK[g]
            pre_rows = sorted(H2ROW[h] for h in range(max(g, 1), HEADS)
                              if QBLK[h] < kb)
            # split into physically-contiguous row runs (h >= g can cut
            # into the middle of a q chunk for odd g)
            run = []
            for r in pre_rows + [None]:
                if run and (r is None or r != run[-1] + 1):
                    units.append({"g": g,
                                  "rows": [PHYS2H[x] for x in run],
                                  "ready": kb})
                    run = []
                if r is not None:
                    run.append(r)
            rest = sorted(h for h in range(max(g, 1), HEADS) if QBLK[h] > kb)
            i = 0
            while i < len(rest):
                h = rest[i]
                pair = [h]
                if i + 1 < len(rest) and QBLK[rest[i + 1]] == QBLK[h]:
                    pair.append(rest[i + 1])
                units.append({"g": g, "rows": pair, "ready": QBLK[h]})
                i += len(pair)
        units.sort(key=lambda u: u["ready"])
        for t in range(T):
            for u in units:
                rows = u["rows"]
                gq = led.gb(w, u["ready"])
                dl = led.gb(w, min(_SCORE_DEADLINE + t, NB - 1))
                if len(rows) <= 2:
                    for h in rows:
                        e, g2 = led.place([(D, CC["sD"])], gq, deadline=dl)
                        put(w, {"k": "sD", "t": t, "h": h, "g": u["g"]}, g2)
                        last_score[(w, t)] = max(
                            last_score.get((w, t), 0), g2)
                else:
                    i = 0
                    while i < len(rows):
                        seg = rows[i:i + _RUNW_S]
                        n = len(seg)
                        if n < 3:
                            for h in seg:
                                e, g2 = led.place([(D, CC["sD"])], gq,
                                                  deadline=dl)
                                put(w, {"k": "sD", "t": t, "h": h,
                                        "g": u["g"]}, g2)
                                last_score[(w, t)] = max(
                                    last_score.get((w, t), 0), g2)
                        else:
                            e, g2 = led.place(
                                [(D, CC["multD"](n)), (PL, CC["multP"](n))],
                                gq, deadline=dl)
                            e3, g3 = led.place([(D, CC["redD"](n))], g2,
                                               deadline=dl)
                            put(w, {"k": "mult", "t": t, "rows": seg,
                                    "g": u["g"], "e": e, "uid": (w, t, u["g"], i)}, g2)
                            put(w, {"k": "red", "t": t, "rows": seg,
                                    "g": u["g"], "uid": (w, t, u["g"], i)}, g3)
                            last_score[(w, t)] = max(
                                last_score.get((w, t), 0), g3)
                        i += n

    # pass 3: exp + softmax per tile
    sm_done = {}
    for w in range(nwin):
        wend = led.gb(w + 1, NB - 1) if w + 1 < nwin else led.n - 1
        for t in range(tiles_per_win[w]):
            r = last_score[(w, t)] + 1
            e, g1 = led.place([(A, CC["exp"])], r, deadline=wend)
            e, g2 = led.place([(D, CC["sm"])], g1, deadline=wend)
            put(w, {"k": "exp", "t": t}, g1)
            put(w, {"k": "softmax", "t": t}, g2)
            sm_done[(w, t)] = g2

    # pass 4: AV (inits column 0, then batched columns 1..15), chains per row
    outblk = {}
    for w in range(nwin):
        T = tiles_per_win[w]
        wend = led.gb(w + 1, NB - 1) if w + 1 < nwin else led.n - 1
        chain = {}
        for t in range(T):
            base = max(sm_done[(w, t)] + 1, led.gb(w, VBLK[0]) + 1)
            e, g0 = led.place([(A, CC["head0"])], base, deadline=wend)
            put(w, {"k": "head0", "t": t}, g0)
            done = g0
            for h in range(1, HEADS):
                e, gi = led.place([(PL, CC["init"]), (A, CC["initA"])], base,
                                  deadline=wend)
                put(w, {"k": "init", "t": t, "h": h, "e": e}, gi)
                chain[(t, h)] = gi
                done = max(done, gi)
            for g in range(1, HEADS):
                vb = led.gb(w, VBLK[g]) + (0 if VBLK[g] == NB - 1 else 1)
                rdy0 = max(sm_done[(w, t)] + 1, vb)
                h = g
                lastwin = (w == nwin - 1) and _LAST_STT
                while h < HEADS:
                    seg = list(range(h, min(h + (1 if lastwin else _RUNW_AV),
                                            HEADS)))
                    n = len(seg)
                    rdy = max([rdy0] + [chain[(t, hh)] for hh in seg])
                    if n == 1 and lastwin:
                        e, gm = led.place(
                            [(D, CC["stt"]), (A, CC["mulA"] + 40.0)], rdy,
                            deadline=wend)
                        if e == D:
                            put(w, {"k": "stt", "t": t, "h": seg[0],
                                    "g": g}, gm)
                        else:
                            uid = (w, t, g, seg[0])
                            led.load(PL, gm, CC["addP"])
                            put(w, {"k": "avmul", "t": t, "h": seg[0],
                                    "g": g, "e": A, "uid": uid, "n": 1,
                                    "j": 0}, gm)
                            put(w, {"k": "avadd_p", "t": t, "rows": seg,
                                    "g": g, "uid": uid}, gm)
                        ga = gm
                    elif n == 1:
                        e, gm = led.place([(D, CC["stt"])], rdy,
                                          deadline=wend)
                        put(w, {"k": "stt", "t": t, "h": seg[0], "g": g}, gm)
                        ga = gm
                    else:
                        muls = []
                        gm = rdy
                        for j, hh in enumerate(seg):
                            e, gmu = led.place(
                                [(PL, CC["mulP"]), (A, CC["mulA"])], rdy,
                                deadline=wend)
                            muls.append((hh, e, gmu))
                            gm = max(gm, gmu)
                        e, ga = led.place([(D, CC["addD"](n))], gm,
                                          deadline=wend)
                        uid = (w, t, g, seg[0])
                        for hh, e2, gmu in muls:
                            put(w, {"k": "avmul", "t": t, "h": hh, "g": g,
                                    "e": e2, "uid": uid, "n": n,
                                    "j": seg.index(hh)}, gmu)
                        put(w, {"k": "avadd", "t": t, "rows": seg, "g": g,
                                "uid": uid}, ga)
                    for hh in seg:
                        chain[(t, hh)] = ga
                    done = max(done, ga)
                    h += n
            g_out = min(done + 1, wend)
            if _OUT_DEFER and w + 1 < nwin:
                g_out = min(max(g_out, led.gb(w + 1, _OUT_DEFER)), wend)
            if _OUTSPLIT:
                done_lo = max([g0] + [chain[(t, hh)] for hh in range(1, 8)])
                g_lo = min(done_lo + 1, wend)
                led.load(A, g_lo, CC["dmaseq"])
                put(w, {"k": "out_lo", "t": t}, g_lo)
                led.load(A, g_out, CC["dmaseq"])
                put(w, {"k": "out_hi", "t": t}, g_out)
            else:
                led.load(A, g_out, CC["dmaseq"])
                put(w, {"k": "out_hi", "t": t}, g_out)
            outblk[(w, t)] = g_out

    return led, sched, copy_eng


# ---------------- emission ----------------


def _emit_op(nc, op, ctx, aux):
    k = op["k"]
    w = op["w"]
    c = ctx[w]
    t = op.get("t")
    trD, trA = aux["trD"], aux["trA"]
    avp, prodp = aux["avp"], aux["prodp"]
    if k == "memset":
        eng = nc.gpsimd if op["e"] == PL else nc.vector
        eng.memset(c["s"][t], -30000.0)
    elif k == "sD":
        h, g = op["h"], op["g"]
        qt = c["qk"][t]
        aux["ti"] += 1
        nc.vector.scalar_tensor_tensor(
            out=trD[aux["ti"] % len(trD)],
            in0=qt[:, QOFS[h]:QOFS[h] + HD],
            scalar=1.0,
            in1=qt[:, QW + g * HD:QW + (g + 1) * HD],
            op0=ALU.bypass, op1=ALU.mult,
            accum_out=c["s"][t][:, H2ROW[h] * HEADS + g:
                                H2ROW[h] * HEADS + g + 1],
        )
    elif k == "mult":
        rows, g = op["rows"], op["g"]
        qt = c["qk"][t]
        n = len(rows)
        r0 = H2ROW[rows[0]]
        prod = prodp.tile([P, _RUNW_S * HD], F16, name="prod", tag="prod")
        aux["prods"][op["uid"]] = prod
        kg = qt[:, QW + g * HD:QW + (g + 1) * HD]
        eng = nc.vector if op["e"] == D else nc.gpsimd
        eng.tensor_tensor(
            out=prod[:, 0:n * HD].rearrange("p (n d) -> p n d", d=HD),
            in0=qt[:, r0 * HD:(r0 + n) * HD].rearrange(
                "p (n d) -> p n d", d=HD),
            in1=kg[:, None, :].to_broadcast((P, n, HD)),
            op=ALU.mult,
        )
    elif k == "red":
        rows, g = op["rows"], op["g"]
        n = len(rows)
        r0 = H2ROW[rows[0]]
        prod = aux["prods"].pop(op["uid"])
        s3 = c["s"][t].rearrange("p (r g) -> p r g", g=HEADS)
        nc.vector.tensor_reduce(
            s3[:, r0:r0 + n, g],
            prod[:, 0:n * HD].rearrange("p (n d) -> p n d", d=HD),
            axis=mybir.AxisListType.X, op=ALU.add,
        )
    elif k == "exp":
        if aux.get("sdbg") is not None and w == 0 and t == 0:
            nc.sync.dma_start(aux["sdbg"], c["s"][t])
        nc.scalar.activation(c["s"][t], c["s"][t], ACTF.Exp)
    elif k == "softmax":
        s3 = c["s"][t].rearrange("p (r g) -> p r g", g=HEADS)
        sumE = c["sm"][t][:, 0:NROW]
        recip = c["sm"][t][:, NROW:2 * NROW]
        nc.vector.tensor_reduce(sumE, s3, axis=mybir.AxisListType.X,
                                op=ALU.add)
        nc.vector.reciprocal(recip, sumE)
        nc.vector.tensor_tensor(
            out=s3, in0=s3,
            in1=recip[:, :, None].to_broadcast((P, NROW, HEADS)),
            op=ALU.mult,
        )
    elif k == "head0":
        nc.scalar.copy(c["acc"][t][:, 0:HD], c["v"][t][:, 0:HD])
    elif k == "init":
        h = op["h"]
        pcol = c["s"][t][:, H2ROW[h] * HEADS:H2ROW[h] * HEADS + 1]
        ah = c["acc"][t][:, h * HD:(h + 1) * HD]
        if op["e"] == PL:
            nc.gpsimd.tensor_scalar_mul(ah, c["v"][t][:, 0:HD], pcol)
        else:
            nc.scalar.activation(ah, c["v"][t][:, 0:HD], ACTF.Copy,
                                 scale=pcol)
    elif k == "stt":
        h, g = op["h"], op["g"]
        pcol = c["s"][t][:, H2ROW[h] * HEADS + g:H2ROW[h] * HEADS + g + 1]
        ah = c["acc"][t][:, h * HD:(h + 1) * HD]
        nc.vector.scalar_tensor_tensor(
            out=ah, in0=c["v"][t][:, g * HD:(g + 1) * HD], scalar=pcol,
            in1=ah, op0=ALU.mult, op1=ALU.add)
    elif k == "avmul":
        h, g, n, j = op["h"], op["g"], op["n"], op["j"]
        uid = op["uid"]
        if uid not in aux["avtmp"]:
            aux["avtmp"][uid] = avp.tile([P, _RUNW_AV * HD], F16,
                                         name="avtmp", tag="avtmp")
        tmp = aux["avtmp"][uid]
        pcol = c["s"][t][:, H2ROW[h] * HEADS + g:H2ROW[h] * HEADS + g + 1]
        vs = c["v"][t][:, g * HD:(g + 1) * HD]
        if op["e"] == PL:
            nc.gpsimd.tensor_scalar_mul(tmp[:, j * HD:(j + 1) * HD], vs, pcol)
        else:
            nc.scalar.activation(tmp[:, j * HD:(j + 1) * HD], vs, ACTF.Copy,
                                 scale=pcol)
    elif k in ("avadd", "avadd_p", "avadd_a"):
        rows = op["rows"]
        n = len(rows)
        h0 = rows[0]
        tmp = aux["avtmp"].pop(op["uid"])
        ar = c["acc"][t][:, h0 * HD:(h0 + n) * HD]
        eng = {"avadd": nc.vector, "avadd_p": nc.gpsimd,
               "avadd_a": None}[k]
        if k == "avadd_a":
            nc.scalar.activation(ar, tmp[:, 0:n * HD], ACTF.Copy,
                                 bias=ar)
        else:
            eng.tensor_tensor(out=ar, in0=ar, in1=tmp[:, 0:n * HD],
                              op=ALU.add)
    elif k in ("out_lo", "out_hi"):
        ti = c["tile0"] + t
        eng = {"A": nc.scalar, "S": nc.sync, "P": nc.gpsimd}[_OUTQ]
        half = DIMS // 2
        if k == "out_hi" and not _OUTSPLIT:
            eng.dma_start(aux["out"][ti * P:(ti + 1) * P, :], c["acc"][t])
        elif k == "out_lo":
            eng.dma_start(aux["out"][ti * P:(ti + 1) * P, 0:half],
                          c["acc"][t][:, 0:half])
        else:
            eng.dma_start(aux["out"][ti * P:(ti + 1) * P, half:DIMS],
                          c["acc"][t][:, half:DIMS])
    else:
        raise ValueError(k)


def _body(tc, xt, wt, out, nt, reps=1, wsizes=None, sdbg=None):
    nc = tc.nc
    if wsizes is None:
        wsizes = _parse_wsizes(nt)
    assert sum(wsizes) == nt, (wsizes, nt)
    nwin_real = len(wsizes)
    nwin = nwin_real * reps
    tiles_per_win = list(wsizes) * reps
    tile0s = []
    o = 0
    for sz in wsizes:
        tile0s.append(o)
        o += sz
    led, sched, copy_eng = plan(nwin, tiles_per_win)
    dsub = DC // WSUB
    with (
        tc.tile_pool(name="xp", bufs=_XP2) as xp,
        tc.tile_pool(name="wp", bufs=_WP2) as wp,
        tc.tile_pool(name="qkp", bufs=TG + 2) as qkp,
        tc.tile_pool(name="vp", bufs=2 * TG) as vp,
        tc.tile_pool(name="psum", bufs=8, space="PSUM") as pp,
        tc.tile_pool(name="sp", bufs=2 * TG) as sp,
        tc.tile_pool(name="smallp", bufs=2 * TG) as smallp,
        tc.tile_pool(name="accp", bufs=TG + 2) as accp,
        tc.tile_pool(name="avp", bufs=_AVPB) as avp,
        tc.tile_pool(name="prodp", bufs=_PRODP2) as prodp,
        tc.tile_pool(name="trashp", bufs=1) as trashp,
    ):
        trD = [trashp.tile([P, HD], F16, name="trash_d", tag=f"trash_d{i}")
               for i in range(8)]
        trA = [trashp.tile([P, HD], F16, name="trash_a", tag=f"trash_a{i}")
               for i in range(8)]
        aux = {"trD": trD, "trA": trA, "avp": avp, "prodp": prodp,
               "out": out, "ti": 0, "prods": {}, "avtmp": {},
               "copy_eng": copy_eng, "sdbg": sdbg}
        ctx = {}
        xeng = {"A": nc.scalar, "S": nc.sync, "P": nc.gpsimd}[_XQ]
        xg_next = [xp.tile([P, DC * P], F16, name="xg", tag="xg")
                   for _ in range(wsizes[0])]
        for t in range(wsizes[0]):
            xeng.dma_start(xg_next[t], xt[t])
        for w in range(nwin):
            gi = w % nwin_real
            T = tiles_per_win[w]
            xg = xg_next
            def _issue_x():
                nonlocal xg_next
                if w + 1 < nwin:
                    gn = (w + 1) % nwin_real
                    xg_next = [xp.tile([P, DC * P], F16, name="xg", tag="xg")
                               for _ in range(tiles_per_win[w + 1])]
                    for t in range(tiles_per_win[w + 1]):
                        xeng.dma_start(xg_next[t], xt[tile0s[gn] + t])
            if _XISSUE == 0:
                _issue_x()
            ctx[w] = {
                "tile0": tile0s[gi],
                "qk": [qkp.tile([P, QW + KW], F16, name="qk", tag="qk")
                       for _ in range(T)],
                "v": [vp.tile([P, DIMS], F16, name="v", tag="v")
                      for _ in range(T)],
                "s": [sp.tile([P, NROW * HEADS], F32, name="s", tag="s")
                      for _ in range(T)],
                "sm": [smallp.tile([P, 2 * NROW], F32, name="sm", tag="sm")
                       for _ in range(T)],
                "acc": [accp.tile([P, DIMS], F16, name="acc", tag="acc")
                        for _ in range(T)],
            }
            if w - 2 in ctx:
                del ctx[w - 2]
            for b in range(NB):
                if _XISSUE and b == _XISSUE:
                    _issue_x()
                kind, heads, wid, wofs, dofs = BLOCKS[b]
                wtile = wp.tile([P, DC, 256], F16, name="w", tag="w")
                for sub in range(WSUB):
                    nc.sync.dma_start(
                        wtile[:, sub * dsub:(sub + 1) * dsub, 0:wid],
                        wt[:, sub * dsub:(sub + 1) * dsub, wofs:wofs + wid],
                    )
                for t in range(T):
                    ps = pp.tile([P, 256], F32, name="ps", tag="ps")
                    for d in range(DC):
                        nc.tensor.matmul(
                            ps[:, 0:wid],
                            lhsT=xg[t][:, d * P:(d + 1) * P],
                            rhs=wtile[:, d, 0:wid],
                            start=(d == 0),
                            stop=(d == DC - 1),
                        )
                    if kind == "v":
                        dst = ctx[w]["v"][t][:, dofs:dofs + wid]
                    else:
                        dst = ctx[w]["qk"][t][:, dofs:dofs + wid]
                    # copies always emitted on planner-chosen engine
                    e = aux.get("copy_eng", {}).get((w, b, t), A)
                    if e == D:
                        nc.vector.tensor_copy(dst, ps[:, 0:wid])
                    elif e == PL:
                        nc.gpsimd.tensor_copy(dst, ps[:, 0:wid])
                    else:
                        nc.scalar.copy(dst, ps[:, 0:wid])
                for op in sched.get(led.gb(w, b), []):
                    _emit_op(nc, op, ctx, aux)
        for g in range(nwin * NB, led.n):
            for op in sched.get(g, []):
                _emit_op(nc, op, ctx, aux)


def build(tpc=TPC, reps=1):
    import concourse.bacc as bacc

    nc = bacc.Bacc(None, target_bir_lowering=False, debug=False,
                   enable_asserts=True, num_devices=NCORES)
    nt = tpc // P
    xt = nc.dram_tensor("xt", [nt, P, DC * P], F16, kind="ExternalInput").ap()
    wt = nc.dram_tensor("wt", [P, DC, WCOLS], F16, kind="ExternalInput").ap()
    out = nc.dram_tensor("out", [tpc, DIMS], F16, kind="ExternalOutput").ap()
    sdbg = None
    if int(os.environ.get("K2_DBG_S", "0")):
        sdbg = nc.dram_tensor("sdbg", [P, NROW * HEADS], F32,
                              kind="ExternalOutput").ap()
    with tile.TileContext(nc) as tc:
        _body(tc, xt, wt, out, nt, reps=reps, sdbg=sdbg)
    nc.compile()
    return nc


def prep_inputs(input_seq_embs, Wq, Wk, Wv, tpc=TPC, ncores=NCORES):
    x = np.asarray(input_seq_embs, dtype=np.float32).reshape(TOK, DIMS)
    Wq = np.asarray(Wq)
    Wk = np.asarray(Wk)
    Wv = np.asarray(Wv)
    rows = []
    for kind, heads, wid, wofs, dofs in BLOCKS:
        Wsrc = {"q": Wq, "k": Wk, "v": Wv}[kind]
        for h in heads:
            rows.append(Wsrc[h * HD:(h + 1) * HD])
    wall = np.concatenate(rows, axis=0).astype(np.float32)  # [WCOLS, DIMS]
    wtile = np.ascontiguousarray(
        wall.T.reshape(DC, P, WCOLS).transpose(1, 0, 2)
    ).astype(np.float16)
    in_maps = []
    nt = tpc // P
    for c in range(ncores):
        xs = x[c * tpc:(c + 1) * tpc]
        xtile = (
            xs.reshape(nt, P, DC, P).transpose(0, 3, 2, 1).astype(np.float16)
        ).reshape(nt, P, DC * P)
        in_maps.append({"xt": np.ascontiguousarray(xtile), "wt": wtile})
    return in_maps


_cached = {}


def _get_nc():
    if "nc" not in _cached:
        _cached["nc"] = build()
    return _cached["nc"]


def kernel_with_results(**inputs):
    from concourse import bass_utils

    nc = _get_nc()
    in_maps = prep_inputs(**inputs)
    res = bass_utils.run_bass_kernel_spmd(
        nc, in_maps, core_ids=list(range(NCORES)), trace=False)
    outs = [r["out"] for r in res.results]
    full = (np.concatenate(outs, axis=0).astype(np.float32)
            .reshape(B, L, DIMS))
    return full, res


def kernel(**inputs):
    return kernel_with_results(**inputs)[0]
